# revision 1
# baseline (speedup 1.0000x reference)
"""Multi-head self-attention on 8 Trainium2 NeuronCores (Bass/Tile) — v2.

Problem: x[4, 2048, 1024], 16 heads x 64 dim, fused QKV/attention/out-proj.

Sharding: core c handles batch b = c//2 and query-half qh = c%2 (1024
queries), all 16 heads. K/V computed for own 1024 tokens and pair-exchanged
(AllGather groups of 2); outputs are disjoint [1024, 1024] slices.

v2 changes vs baseline:
  - inner loop pipelined: scores emitted one kc ahead and PV trails exp by
    one kc, so TensorE's strict FIFO never head-of-line blocks on the
    current exp; ScalarE runs exps back-to-back
  - softmax accumulated pair-wise ([128, 2048] adds, 7 instead of 15)
  - exp split between ScalarE (Exp activation) and DVE (Schraudolph int16
    bit-trick: round(s*1024/ln2 + (15 - C)*1024) bitcast as fp16)
  - Q/K projection PSUM evacuation moved to ScalarE (Identity + bias)
  - o_proj bias applied on host (free epilogue), evac is a plain copy
  - K bias dropped: softmax is exactly invariant to it ((q+qb)@kb is
    constant over k; exp factor cancels between numerator and denominator)
"""

import numpy as np

EMBED = 1024
NH = 16
D = 64
B = 4
T = 2048
TQ = 1024  # queries per core
NCORES = 8
NIC = EMBED // 128  # 8 contraction chunks
NHP = NH // 2  # 8 head pairs

# Schraudolph fp16 fake-exp constants; input st = s/8 scores
_A16 = 1024.0 / np.log(2.0)
_C16 = 0.0450465
_B16 = 15.0 * 1024.0 - _C16 * 1024.0
# which kc's exp run on DVE instead of ScalarE (per (hp, qb), of 16)
DSET = (5, 11)

_PROGRAM = None


def _build_program():
    import concourse.bass as bass
    import concourse.mybir as mybir
    import concourse.tile as tile
    from concourse import bacc

    F16 = mybir.dt.float16
    F32 = mybir.dt.float32
    I16 = mybir.dt.int16
    AF = mybir.ActivationFunctionType
    ALU = mybir.AluOpType

    nc = bacc.Bacc("TRN2", target_bir_lowering=False, debug=False,
                   num_devices=NCORES)

    xT_d = nc.dram_tensor("xT", [EMBED, T], F16, kind="ExternalInput").ap()
    wq_d = nc.dram_tensor("wqT", [EMBED, EMBED], F16, kind="ExternalInput").ap()
    wk_d = nc.dram_tensor("wkT", [EMBED, EMBED], F16, kind="ExternalInput").ap()
    wv_d = nc.dram_tensor("wvT", [EMBED, EMBED], F16, kind="ExternalInput").ap()
    wo_d = nc.dram_tensor("woT", [EMBED, EMBED], F16, kind="ExternalInput").ap()
    qb_d = nc.dram_tensor("qb", [128, NIC], F32, kind="ExternalInput").ap()
    vbb_d = nc.dram_tensor("vbb", [128, EMBED], F16, kind="ExternalInput").ap()
    ones_d = nc.dram_tensor("ones", [128, 1], F16, kind="ExternalInput").ap()
    sel_d = nc.dram_tensor("sel", [1, 256], F16, kind="ExternalInput").ap()
    y_d = nc.dram_tensor("y", [TQ, EMBED], F32, kind="ExternalOutput").ap()

    # pair-exchange staging (replica groups of 2)
    PAIRS = [[2 * g, 2 * g + 1] for g in range(NCORES // 2)]
    kxb_d = [nc.dram_tensor(f"kxb{h}", [128, TQ], F16).ap() for h in range(NHP)]
    kg_d = [nc.dram_tensor(f"kg{h}", [2, 128, TQ], F16).ap()
            for h in range(NHP)]
    ccw_in = nc.dram_tensor("ccw_in", [1, 64], F16).ap()
    ccw_out = nc.dram_tensor("ccw_out", [2, 64], F16).ap()
    vxb_d = [[nc.dram_tensor(f"vxb{o}h{h}", [512, 512], F16).ap()
              for h in range(2)] for o in range(2)]
    vg_d = [[nc.dram_tensor(f"vg{o}h{h}", [2, 512, 512], F16).ap()
             for h in range(2)] for o in range(2)]

    xT_r = xT_d.rearrange("(c p) t -> c p t", p=128)
    wq_r = wq_d.rearrange("(c p) o -> p c o", p=128)
    wk_r = wk_d.rearrange("(c p) o -> p c o", p=128)
    wv_r = wv_d.rearrange("(c p) o -> c p o", p=128)
    wo_r = wo_d.rearrange("(c p) o -> c p o", p=128)
    y_r = y_d.rearrange("(tb p) o -> tb p o", p=128)

    NKC = T // 128       # 16 key chunks
    NQB = TQ // 512      # 2 query blocks
    NTB = T // 128       # 16 token blocks for V

    with tile.TileContext(nc) as tc:
        from contextlib import ExitStack
        with ExitStack() as ctx:
            cst = ctx.enter_context(tc.tile_pool(name="cst", bufs=1))
            big = ctx.enter_context(tc.tile_pool(name="big", bufs=1))
            wqk = ctx.enter_context(tc.tile_pool(name="wqk", bufs=3))
            qkp = ctx.enter_context(tc.tile_pool(name="qkp", bufs=3))
            pTp = ctx.enter_context(tc.tile_pool(name="pTp", bufs=4))
            accp = ctx.enter_context(tc.tile_pool(name="accp", bufs=2))
            misc = ctx.enter_context(tc.tile_pool(name="misc", bufs=2))
            outp = ctx.enter_context(tc.tile_pool(name="outp", bufs=3))
            ps_st = ctx.enter_context(
                tc.tile_pool(name="ps_st", bufs=2, space="PSUM"))
            ps_pv = ctx.enter_context(
                tc.tile_pool(name="ps_pv", bufs=2, space="PSUM"))
            ps_sm = ctx.enter_context(
                tc.tile_pool(name="ps_sm", bufs=2, space="PSUM"))

            # ---- persistent tiles ----
            xT = big.tile([128, NIC * T], F16, tag="xT")          # 32KB/par
            v0l = big.tile([128, NTB * 128], F16, tag="v0l")      # 4KB
            wv = big.tile([128, NIC * EMBED], F16, tag="wv")      # 16KB
            wo = big.tile([128, NIC * EMBED], F16, tag="wo")      # 16KB
            vv = big.tile([128, NTB * EMBED], F16, tag="vv")      # 32KB
            aT = big.tile([128, NHP * TQ], F16, tag="aT")         # 16KB
            qb_sb = cst.tile([128, NIC], F32, tag="qb")
            vbb = cst.tile([128, EMBED], F16, tag="vbb")
            ones = cst.tile([128, 1], F16, tag="ones")
            sel = cst.tile([1, 256], F16, tag="sel")

            for c in range(NIC):
                nc.sync.dma_start(xT[:, c * T: c * T + 512],
                                  xT_r[c][:, 0:512])
            wq0_sb = wqk.tile([128, NIC * 128], F16, tag="wq", name="wq0")
            wk0_sb = wqk.tile([128, NIC * 128], F16, tag="wk", name="wk0")
            nc.sync.dma_start(
                wq0_sb[:].rearrange("p (c o) -> p c o", c=NIC),
                wq_r[:, :, 0:128])
            nc.sync.dma_start(
                wk0_sb[:].rearrange("p (c o) -> p c o", c=NIC),
                wk_r[:, :, 0:128])
            for c in range(NIC):
                nc.sync.dma_start(wv[:, c * EMBED: c * EMBED + 128],
                                  wv_r[c][:, 0:128])
            nc.sync.dma_start(qb_sb[:], qb_d[:])
            nc.sync.dma_start(ones[:], ones_d[:])
            nc.sync.dma_start(sel[:], sel_d[:])
            nc.sync.dma_start(vbb[:], vbb_d[:])
            # warm the PE clock gate (HAM) with a dummy MM burst that has
            # no DMA dependency: ~3.5us of activity flips K=4/8 -> 8/8
            # before the real startup units arrive (~t=5us)
            scr = misc.tile([128, 512], F16, tag="scr", name="scr")
            nc.vector.memset(scr[:], 0.0)
            wps = ps_sm.tile([128, 512], F32, tag="small", name="wps")
            for _ in range(18):
                nc.tensor.matmul(wps[:], lhsT=scr[:, 0:128], rhs=scr[:],
                                 start=True, stop=True)
            # warm the exp activation table while DMAs stream
            warm = misc.tile([128, 1], F16, tag="warm")
            nc.scalar.activation(warm[:], ones[:], AF.Exp)

            # ---------- projection machinery ----------
            def v_proj_tb(ob, tb):
                # V[tok, feat] for OWN token block tb (<8), feat block ob*512
                ps = ps_sm.tile([128, 512], F32, tag="small")
                for c in range(NIC):
                    nc.tensor.matmul(
                        ps[:],
                        lhsT=xT[:, c * T + tb * 128: c * T + tb * 128 + 128],
                        rhs=wv[:, c * EMBED + ob * 512: c * EMBED + ob * 512 + 512],
                        start=(c == 0), stop=(c == NIC - 1))

                def evac():
                    vtmp = outp.tile([128, 512], F16, tag="vtmp")
                    nc.vector.tensor_add(vtmp[:], ps[:],
                                         vbb[:, ob * 512:(ob + 1) * 512])
                    nc.gpsimd.dma_start(
                        vxb_d[ob][tb // 4][(tb % 4) * 128:(tb % 4) * 128 + 128, :],
                        vtmp[:])
                return evac

            def v_gather(ob, half):
                nc.gpsimd.collective_compute(
                    "AllGather", mybir.AluOpType.bypass,
                    ins=[vxb_d[ob][half][:]], outs=[vg_d[ob][half][:]],
                    replica_groups=PAIRS)
                for g in range(2):
                    for j in range(4):
                        gt = g * 8 + half * 4 + j
                        nc.gpsimd.dma_start(
                            vv[:, gt * EMBED + ob * 512:
                               gt * EMBED + ob * 512 + 512],
                            vg_d[ob][half][g, j * 128:(j + 1) * 128, :])

            kq = {}

            def alloc_kq(hp):
                wq_sb = wqk.tile([128, NIC * 128], F16, tag="wq")
                wk_sb = wqk.tile([128, NIC * 128], F16, tag="wk")
                nc.sync.dma_start(
                    wq_sb[:].rearrange("p (c o) -> p c o", c=NIC),
                    wq_r[:, :, hp * 128:(hp + 1) * 128])
                nc.sync.dma_start(
                    wk_sb[:].rearrange("p (c o) -> p c o", c=NIC),
                    wk_r[:, :, hp * 128:(hp + 1) * 128])
                kT = qkp.tile([128, T], F16, tag="kT")
                qT = qkp.tile([128, TQ], F16, tag="qT")
                if hp:
                    kTh = qkp.tile([128, TQ], F16, tag="kTh")
                else:
                    kTh = None
                kq[hp] = (wq_sb, wk_sb, kT, qT, kTh)

            kq_ps = {}

            def kq_unit(hp, tb, which, half=None):
                # 8 MMs (or a 4-MM half); evacuation returned as a closure
                # to be emitted ~2 slots later (None for the h0 half).
                # which: 'q' (tb<2), 'k' own tokens (tb<2), 'k0' full (tb<4)
                wq_sb, wk_sb, kT, qT, kTh = kq[hp]
                key = (hp, tb, which)
                if half in (None, 0):
                    kq_ps[key] = ps_sm.tile([128, 512], F32, tag="small",
                                            name="ps")
                ps = kq_ps[key]
                w_sb = wq_sb if which == "q" else wk_sb
                crange = range(NIC) if half is None else \
                    range(4 * half, 4 * half + 4)
                for c in crange:
                    nc.tensor.matmul(
                        ps[:], lhsT=w_sb[:, c * 128:(c + 1) * 128],
                        rhs=xT[:, c * T + tb * 512: c * T + tb * 512 + 512],
                        start=(c == 0), stop=(c == NIC - 1))
                if half == 0:
                    return None

                def evac():
                    if which == "q":
                        nc.scalar.add(qT[:, tb * 512:(tb + 1) * 512], ps[:],
                                      qb_sb[:, hp:hp + 1])
                    elif which == "k0":
                        nc.scalar.copy(kT[:, tb * 512:(tb + 1) * 512], ps[:])
                    else:
                        nc.scalar.copy(kTh[:, tb * 512:(tb + 1) * 512],
                                       ps[:])
                return evac

            def v_proj0_tb(tb):
                # hp0: V[tok, first 128 feats] over ALL tokens locally
                ps = ps_sm.tile([128, 512], F32, tag="small")
                for c in range(NIC):
                    nc.tensor.matmul(
                        ps[:, 0:128],
                        lhsT=xT[:, c * T + tb * 128: c * T + tb * 128 + 128],
                        rhs=wv[:, c * EMBED: c * EMBED + 128],
                        start=(c == 0), stop=(c == NIC - 1))
                def evac():
                    nc.vector.tensor_add(v0l[:, tb * 128:(tb + 1) * 128],
                                         ps[:, 0:128], vbb[:, 0:128])
                return evac

            def k_gather(hp):
                _, _, kT, _, kTh = kq[hp]
                nc.gpsimd.dma_start(kxb_d[hp][:], kTh[:])
                nc.gpsimd.collective_compute(
                    "AllGather", mybir.AluOpType.bypass,
                    ins=[kxb_d[hp][:]], outs=[kg_d[hp][:]],
                    replica_groups=PAIRS)
                nc.gpsimd.dma_start(kT[:, 0:TQ], kg_d[hp][0])
                nc.gpsimd.dma_start(kT[:, TQ:T], kg_d[hp][1])

            def o_unit(tb, ob, half=None):
                key = ("o", tb, ob)
                if half in (None, 0):
                    kq_ps[key] = ps_sm.tile([128, 512], F32, tag="small",
                                            name="ps")
                ps = kq_ps[key]
                frange = range(NHP) if half is None else \
                    range(4 * half, 4 * half + 4)
                for f in frange:
                    nc.tensor.matmul(
                        ps[:],
                        lhsT=aT[:, f * TQ + tb * 128: f * TQ + tb * 128 + 128],
                        rhs=wo[:, f * EMBED + ob * 512:
                               f * EMBED + ob * 512 + 512],
                        start=(f == 0), stop=(f == NHP - 1))
                if half == 0:
                    return None

                def evac():
                    out_sb = outp.tile([128, 512], F32, tag="out")
                    nc.vector.tensor_copy(out_sb[:], ps[:])
                    nc.gpsimd.dma_start(y_r[tb][:, ob * 512:(ob + 1) * 512],
                                        out_sb[:])
                return evac

            # ---------- startup ----------
            kT0 = qkp.tile([128, T], F16, tag="kT", name="kT0")
            qT0 = qkp.tile([128, TQ], F16, tag="qT", name="qT0")
            kq[0] = (wq0_sb, wk0_sb, kT0, qT0, None)
            for c in range(NIC):
                nc.sync.dma_start(xT[:, c * T + 512: c * T + TQ],
                                  xT_r[c][:, 512:TQ])
            alloc_kq(1)
            for c in range(NIC):
                nc.sync.dma_start(xT[:, c * T + TQ:(c + 1) * T],
                                  xT_r[c][:, TQ:T])
            for c in range(NIC):
                nc.sync.dma_start(wv[:, c * EMBED + 128:(c + 1) * EMBED],
                                  wv_r[c][:, 128:EMBED])
            kq_unit(0, 0, "k0")()
            kq_unit(0, 0, "q")()
            v_proj0_tb(0)()
            v_proj0_tb(1)()
            kq_unit(0, 1, "k0")()
            kq_unit(0, 2, "k0")()
            kq_unit(0, 3, "k0")()
            kq_unit(1, 0, "k")()
            kq_unit(1, 1, "k")()
            k_gather(1)
            evs0 = {}
            for c in range(NIC):
                nc.sync.dma_start(wo[:, c * EMBED:(c + 1) * EMBED], wo_r[c])

            # ---------- attention main loop ----------
            pending_tail = [None]
            pending = [[]]

            def flush_tail():
                if pending_tail[0] is None:
                    return
                hp_, qb_, pv_, acc_ = pending_tail[0]
                pending_tail[0] = None
                # softmax denominators from pair-shaped acc [128, 2048]:
                # head h sums = ones^T @ (acc[:, h*512:+512] + acc[:, 1024+h*512:+512])
                sums = ps_sm.tile([128, 512], F32, tag="small")
                bc = sums
                nc.tensor.matmul(sums[0:1, :], lhsT=ones[:],
                                 rhs=acc_[:, 0:512], start=True, stop=False)
                nc.tensor.matmul(sums[0:1, :], lhsT=ones[:],
                                 rhs=acc_[:, 1024:1536], start=False,
                                 stop=True)
                nc.tensor.matmul(sums[32:33, :], lhsT=ones[:],
                                 rhs=acc_[:, 512:1024], start=True,
                                 stop=False, tile_position=(0, 32))
                nc.tensor.matmul(sums[32:33, :], lhsT=ones[:],
                                 rhs=acc_[:, 1536:2048], start=False,
                                 stop=True, tile_position=(0, 32))
                sums_sb = misc.tile([1, 1024], F16, tag="sums_sb")
                with nc.allow_low_precision(
                        reason="softmax denominators, fp16 ample"):
                    nc.vector.tensor_copy(
                        sums_sb[:, 0:512], sums[0:1, 0:512])
                    nc.vector.tensor_copy(
                        sums_sb[:, 512:1024], sums[32:33, 0:512])
                nc.tensor.matmul(bc[:], lhsT=sel[:, 0:128],
                                 rhs=sums_sb[:, 0:512], start=True,
                                 stop=False)
                nc.tensor.matmul(bc[:], lhsT=sel[:, 128:256],
                                 rhs=sums_sb[:, 512:1024], start=False,
                                 stop=True)
                bc_sb = misc.tile([128, 512], F32, tag="bc_sb")
                nc.vector.reciprocal_approx_fast(bc_sb[:], bc[:])
                nc.vector.tensor_mul(
                    aT[:, hp_ * TQ + qb_ * 512: hp_ * TQ + qb_ * 512 + 512],
                    pv_[:], bc_sb[:])

            def pv_mms(hp, kc, pv, pair, kcstop):
                half = (kc % 2) * 1024
                if hp == 0:
                    vl0 = v0l[:, kc * 128: kc * 128 + 64]
                    vl1 = v0l[:, kc * 128 + 64: kc * 128 + 128]
                else:
                    vl0 = vv[:, kc * EMBED + hp * 128:
                             kc * EMBED + hp * 128 + 64]
                    vl1 = vv[:, kc * EMBED + hp * 128 + 64:
                             kc * EMBED + hp * 128 + 128]
                nc.tensor.matmul(
                    pv[0:64, :], lhsT=vl0, rhs=pair[:, half: half + 512],
                    start=(kc == 0), stop=kcstop)
                nc.tensor.matmul(
                    pv[64:128, :], lhsT=vl1,
                    rhs=pair[:, half + 512: half + 1024],
                    start=(kc == 0), stop=kcstop,
                    tile_position=(0, 64))

            for hp in range(NHP):
                _, _, kT, qT, _ = kq[hp]

                for qb in range(NQB):
                    if hp + 2 < NHP and qb == 0:
                        alloc_kq(hp + 2)
                    # units[kc] = [(fn, args, evac_delta)]
                    units = {}

                    def add(kc_, fn, args, delta=1):
                        units.setdefault(kc_, []).append((fn, args, delta))

                    if hp == 0 and qb == 0:
                        add(1, v_proj0_tb, (2,))
                        add(1, v_proj0_tb, (3,))
                        for tb_ in range(4, 16):
                            add(tb_ - 2, v_proj0_tb, (tb_,))
                        add(14, kq_unit, (0, 1, "q"), 1)
                        add(15, v_proj_tb, (0, 0))
                    elif hp == 0 and qb == 1:
                        for j in range(3):
                            add(1 + j, v_proj_tb, (0, 1 + j))
                        add(4, v_gather, (0, 0))
                        for j in range(4):
                            add(5 + j, v_proj_tb, (0, 4 + j))
                        add(9, v_gather, (0, 1))
                        add(10, kq_unit, (1, 0, "q", 0))
                        add(11, kq_unit, (1, 0, "q", 1))
                        add(12, kq_unit, (1, 1, "q", 0))
                        add(13, kq_unit, (1, 1, "q", 1))
                        add(14, kq_unit, (2, 0, "k", 0))
                        add(15, kq_unit, (2, 0, "k", 1))
                    elif hp == 1 and qb == 0:
                        add(1, kq_unit, (2, 1, "k", 0))
                        add(2, kq_unit, (2, 1, "k", 1), 2)
                        add(5, k_gather, (2,))
                        add(4, v_proj_tb, (1, 0))
                        add(6, v_proj_tb, (1, 1))
                        add(8, v_proj_tb, (1, 2))
                        add(10, v_proj_tb, (1, 3))
                        add(11, v_gather, (1, 0))
                        add(12, kq_unit, (3, 0, "k", 0))
                        add(13, kq_unit, (3, 0, "k", 1), 2)
                        add(14, kq_unit, (3, 1, "k", 0))
                        add(15, kq_unit, (3, 1, "k", 1), 2)
                    elif hp == 1 and qb == 1:
                        add(1, v_proj_tb, (1, 4))
                        add(3, v_proj_tb, (1, 5))
                        add(5, v_proj_tb, (1, 6))
                        add(8, v_proj_tb, (1, 7))
                        add(12, v_gather, (1, 1))
                        add(6, k_gather, (3,))
                        add(9, kq_unit, (2, 0, "q", 0))
                        add(10, kq_unit, (2, 0, "q", 1), 2)
                        add(13, kq_unit, (2, 1, "q", 0))
                        add(14, kq_unit, (2, 1, "q", 1), 1)
                    elif qb == 0:
                        if hp + 2 < NHP:
                            add(1, kq_unit, (hp + 2, 0, "k", 0))
                            add(2, kq_unit, (hp + 2, 0, "k", 1), 2)
                            add(5, kq_unit, (hp + 2, 1, "k", 0))
                            add(6, kq_unit, (hp + 2, 1, "k", 1), 2)
                    else:
                        if hp + 2 < NHP:
                            add(7, k_gather, (hp + 2,))
                        if hp + 1 < NHP:
                            add(3, kq_unit, (hp + 1, 0, "q", 0))
                            add(4, kq_unit, (hp + 1, 0, "q", 1), 2)
                            add(10, kq_unit, (hp + 1, 1, "q", 0))
                            add(11, kq_unit, (hp + 1, 1, "q", 1), 2)
                        else:
                            for u in range(7):
                                tb_, ob_ = u // 2, u % 2
                                add(2 + 2 * u, o_unit, (tb_, ob_, 0))
                                add(3 + 2 * u, o_unit, (tb_, ob_, 1), 2)

                    def scores(kc):
                        st = ps_st.tile([128, 1024], F32, tag="st",
                                        name="st")
                        nc.tensor.matmul(
                            st[:, 0:512],
                            lhsT=kT[0:64, kc * 128:(kc + 1) * 128],
                            rhs=qT[0:64, qb * 512:(qb + 1) * 512],
                            start=True, stop=True)
                        nc.tensor.matmul(
                            st[:, 512:1024],
                            lhsT=kT[64:128, kc * 128:(kc + 1) * 128],
                            rhs=qT[64:128, qb * 512:(qb + 1) * 512],
                            start=True, stop=True, tile_position=(64, 0))
                        return st

                    pv = ps_pv.tile([128, 512], F32, tag="pv")
                    acc = accp.tile([128, 2048], F16, tag="acc")
                    pairs = {}
                    pending_evacs = pending[0]
                    evs = evs0 if (hp == 0 and qb == 0) else {}
                    sts = {0: scores(0)}
                    for kc in range(NKC):
                        # 1. exp -> pair half (ScalarE or DVE fake-exp)
                        p = kc // 2
                        if kc % 2 == 0:
                            pairs[p] = pTp.tile([128, 2048], F16, tag="pT", name="pT")
                        pair = pairs[p]
                        half = (kc % 2) * 1024
                        dst = pair[:, half: half + 1024]
                        st = sts.pop(kc)
                        with nc.allow_low_precision(
                                reason="fp16 softmax weights"):
                            if kc in DSET:
                                nc.vector.tensor_scalar(
                                    dst.bitcast(I16), st[:], _A16, _B16,
                                    ALU.mult, ALU.add)
                            else:
                                nc.scalar.activation(dst, st[:], AF.Exp)
                        # scheduled unit evacuations (incl. carried-over)
                        if kc == 0:
                            for ev in pending_evacs:
                                ev()
                            pending_evacs.clear()
                        for ev in evs.pop(kc, []):
                            ev()
                        # 2. PV for previous kc
                        if kc >= 1:
                            pv_mms(hp, kc - 1, pv, pairs[(kc - 1) // 2],
                                   False)
                        # 3. next scores (st double buffer frees after the
                        #    exp of kc-1, so TensorE won't block)
                        if kc + 1 < NKC:
                            sts[kc + 1] = scores(kc + 1)
                        # 4. pair-wise softmax accumulation (DVE)
                        with nc.allow_low_precision(
                                reason="fp16 softmax partial sums"):
                            if kc == 4:
                                nc.vector.tensor_add(
                                    acc[:], pairs[0][:], pairs[1][:])
                            elif kc >= 6 and kc % 2 == 0:
                                nc.vector.tensor_add(
                                    acc[:], acc[:], pairs[kc // 2 - 1][:])
                        # 5. deferred softmax tail of the previous (hp, qb)
                        if kc == 2:
                            flush_tail()
                        # 6. interleaved projection work (evacs deferred to
                        #    the next iteration, emitted right after exp)
                        if kc in units:
                            for fn, args, delta in units[kc]:
                                ev = fn(*args)
                                if ev is not None:
                                    if kc + delta < NKC:
                                        evs.setdefault(
                                            kc + delta, []).append(ev)
                                    else:
                                        pending_evacs.append(ev)
                    # loop epilogue (carried evacs stay for next qb's kc0)
                    for k_ in sorted(evs):
                        for ev in evs.pop(k_):
                            ev()
                    pv_mms(hp, NKC - 1, pv, pairs[7], True)
                    with nc.allow_low_precision(
                            reason="fp16 softmax partial sums"):
                        nc.vector.tensor_add(acc[:], acc[:], pairs[7][:])
                    pending_tail[0] = (hp, qb, pv, acc)

            for ev in pending[0]:
                ev()
            pending[0].clear()

            # ---- remaining out projection (qb1 token blocks) ----
            # two tail units pre-compute chunks f=0..6 (only f=7 needs the
            # final flush's aT), overlapping the flush chain
            def o_partial(tb, ob):
                ps = ps_sm.tile([128, 512], F32, tag="small", name="ps")
                for f in range(NHP - 1):
                    nc.tensor.matmul(
                        ps[:],
                        lhsT=aT[:, f * TQ + tb * 128: f * TQ + tb * 128 + 128],
                        rhs=wo[:, f * EMBED + ob * 512:
                               f * EMBED + ob * 512 + 512],
                        start=(f == 0), stop=False)

                def finish():
                    f = NHP - 1
                    nc.tensor.matmul(
                        ps[:],
                        lhsT=aT[:, f * TQ + tb * 128: f * TQ + tb * 128 + 128],
                        rhs=wo[:, f * EMBED + ob * 512:
                               f * EMBED + ob * 512 + 512],
                        start=False, stop=True)
                    out_sb = outp.tile([128, 512], F32, tag="out")
                    nc.vector.tensor_copy(out_sb[:], ps[:])
                    nc.gpsimd.dma_start(y_r[tb][:, ob * 512:(ob + 1) * 512],
                                        out_sb[:])
                return finish

            fin_a = o_partial(3, 1)
            flush_tail()
            fin_a()
            o_unit(4, 0, 0)
            o_unit(4, 0, 1)()
            o_unit(4, 1, 0)
            o_unit(4, 1, 1)()
            for tb in range(5, TQ // 128):
                for ob in range(2):
                    o_unit(tb, ob)()

    nc.compile()
    return nc


def _get_program():
    global _PROGRAM
    if _PROGRAM is None:
        _PROGRAM = _build_program()
    return _PROGRAM


def _make_in_maps(x, q_w, q_b, k_w, k_b, v_w, v_b, o_w, o_b):
    f16 = np.float16
    # softmax scale folded into the Q projection
    wqT = np.ascontiguousarray((q_w.astype(np.float32).T / 8.0)).astype(f16)
    wkT = np.ascontiguousarray(k_w.astype(np.float32).T).astype(f16)
    wvT = np.ascontiguousarray(v_w.astype(np.float32).T).astype(f16)
    woT = np.ascontiguousarray(o_w.astype(np.float32).T).astype(f16)
    qb = np.ascontiguousarray(
        (q_b.astype(np.float32) / 8.0).reshape(NIC, 128).T)
    vbb = np.broadcast_to(v_b.astype(np.float32), (128, EMBED)).astype(f16)
    vbb = np.ascontiguousarray(vbb)
    ones = np.ones((128, 1), f16)
    sel = np.zeros((1, 256), f16)
    sel[0, 0:64] = 1.0
    sel[0, 192:256] = 1.0
    in_maps = []
    for c in range(NCORES):
        b, qh = c // 2, c % 2
        xb = x[b].astype(np.float32)
        xp = np.concatenate(
            [xb[qh * TQ:(qh + 1) * TQ], xb[(1 - qh) * TQ:(2 - qh) * TQ]],
            axis=0)
        xT = np.ascontiguousarray(xp.T).astype(f16)
        in_maps.append({
            "xT": xT, "wqT": wqT, "wkT": wkT, "wvT": wvT, "woT": woT,
            "qb": qb, "vbb": vbb, "ones": ones, "sel": sel,
        })
    return in_maps


def kernel(x, mask, q_w, q_b, k_w, k_b, v_w, v_b, o_w, o_b):
    from concourse.bass_utils import run_bass_kernel_spmd

    nc = _get_program()
    x = np.asarray(x)
    in_maps = _make_in_maps(np.asarray(x), np.asarray(q_w), np.asarray(q_b),
                            np.asarray(k_w), np.asarray(k_b),
                            np.asarray(v_w), np.asarray(v_b),
                            np.asarray(o_w), np.asarray(o_b))
    res = run_bass_kernel_spmd(nc, in_maps, list(range(NCORES)))
    ob = np.asarray(o_b, np.float32)
    out = np.empty((B, T, EMBED), np.float32)
    for c in range(NCORES):
        b, qh = c // 2, c % 2
        out[b, qh * TQ:(qh + 1) * TQ, :] = res.results[c]["y"] + ob
    return out

